# revision 37
# baseline (speedup 1.0000x reference)
"""CTPN loss kernel for 8 Trainium2 NeuronCores — dense-slab design.

Strategy (data parallel over positions, no GPSIMD custom ops):
  * The H*W=24576 spatial positions are split into 8 contiguous slices of
    3072; core c holds all 50 map channels for its slice in bf16.
  * Smooth-L1 terms are evaluated DENSELY over the vertical/side maps: the
    host builds a target grid TM that defaults to the map values themselves,
    so un-referenced cells contribute exactly 0, and writes the regression
    target at each referenced cell (duplicate references spill into a small
    host-filled overflow region).  Using
        sl1(d) = 0.5*d^2 - 0.5*(d - clamp(d,-1,1))^2
    the whole reduction is two ACT Square+accumulate passes plus three DVE
    passes (subtract, fused clamp, subtract) — no gather.
  * Classification CE is likewise dense: the score maps are laid out as
    (first, second) logit pairs per anchor-position — the host pre-swaps the
    pair order for negative-labelled cells — and a per-cell bf16 count grid
    weights softplus(first - second); CE = sum(W * softplus(D)).
  * Per-partition partial sums return to the host, which applies the
    divisors (1/(2*Nv), 1/No, 1/Ns) and sums across cores (the all-reduce).
  * All activations (Square, Softplus) live in one ACT table, loaded once
    off the critical path while the input DMAs stream.
"""

import sys

sys.path.insert(0, "/opt/trn_rl_repo")

import numpy as np

import concourse.bacc as bacc
from concourse import mybir
from concourse import bass_utils

# ---------------- problem constants (hardcoded per contract) ----------------
H, W, K = 128, 192, 10
HW = H * W                      # 24576
N_CORES = 8
PPC = HW // N_CORES             # 3072 positions per core
NS = 128.0
NV_REG = 20000
NO_REG = 5000

F = 768                         # slab free dim (elements per partition)
VP_PARTS = 80                   # 20 ch * 3072 / 768
SD_PARTS = 40                   # 10 ch * 3072 / 768
OV0 = VP_PARTS + SD_PARTS       # overflow partitions 120..127
NPAIR = 240                     # score pairs per partition (10*3072/128)
OVP = 16                        # overflow pair slots per partition
NPW = NPAIR + OVP               # W grid width = 256
SCW = 2 * NPW                   # score cols = 512

o_data = 0
o_tm = 2 * F                    # 1536
o_sc = 4 * F                    # 3072
WB = o_sc + 2 * SCW             # 4096 bytes per partition

_cache = {}


def _bf16(x):
    """Round f32 -> bf16 (RNE), return uint16 bit patterns."""
    u = np.asarray(x, np.float32).view(np.uint32)
    r = (u + 0x7FFF + ((u >> 16) & 1)) >> 16
    return r.astype(np.uint16)


def _build_bass():
    """Raw-bass kernel (no TileContext).

    With manual semaphores there is no context-exit barrier: each engine
    falls through to the compiler-emitted fini (its share of the 256-sem
    clear storm, ~2-7us) as soon as its own stream ends.  The idle engines
    (Tensor, GpSimd) therefore run their fini clears DURING the body
    instead of after it.  A small gate keeps GpSimd — whose clear chunk
    covers our live semaphores — from clearing them while in use.
    """
    nc = bacc.Bacc("TRN2", target_bir_lowering=False)
    MEGA = nc.dram_tensor("mega", [128, WB], mybir.dt.uint8, kind="ExternalInput")
    OUT = nc.dram_tensor("out", [128, 4], mybir.dt.float32, kind="ExternalOutput")

    f32 = mybir.dt.float32
    bf16 = mybir.dt.bfloat16
    AF = mybir.ActivationFunctionType

    MEG = nc.alloc_sbuf_tensor("megs", [128, WB], mybir.dt.uint8)
    P = nc.alloc_sbuf_tensor("P", [128, 4], f32)
    dm = nc.alloc_sbuf_tensor("dmT", [128, F], bf16)
    c = nc.alloc_sbuf_tensor("cT", [128, F], bf16)
    dc = nc.alloc_sbuf_tensor("dcT", [128, F], bf16)
    sq = nc.alloc_sbuf_tensor("sqT", [128, F], bf16)
    D = nc.alloc_sbuf_tensor("DT", [128, NPW], f32)
    ex = nc.alloc_sbuf_tensor("exT", [128, NPW], f32)
    ce = nc.alloc_sbuf_tensor("ceT", [128, NPW], f32)

    SA = nc.alloc_semaphore("in_slab")
    SB = nc.alloc_semaphore("in_cls")
    S1 = nc.alloc_semaphore("dve_ms")
    S2 = nc.alloc_semaphore("act_ms")
    SO = nc.alloc_semaphore("out_done")
    SGATE = nc.alloc_semaphore("out_issued")

    data_v = MEG[:, o_data:o_tm].bitcast(bf16)   # [128, 768]
    tm_v = MEG[:, o_tm:o_sc].bitcast(bf16)       # [128, 768]
    sc_v = MEG[:, o_sc:WB].bitcast(bf16)         # [128, 512]

    # SP: both input DMAs on one queue; slab first
    nc.sync.dma_start(MEG[:, o_data:o_sc], MEGA[:, o_data:o_sc]).then_inc(SA, 16)
    nc.sync.dma_start(MEG[:, o_sc:WB], MEGA[:, o_sc:WB]).then_inc(SB, 16)

    # ACT: the single table load (natural_log_exp_and_others, id 6:
    # Square+Exp+Ln) overlaps the input DMAs and suppresses auto-loads
    nc.scalar.add_instruction(mybir.InstLoadActFuncSet(
        name=nc.get_next_instruction_name(), ins=[], outs=[],
        act_func_set_id=6))

    # DVE queue.  smooth-l1 via  sum sl1(d) = sum d*c - 0.5*sum c^2,
    # c = clamp(d, -1, 1);  d = v - t is 0 on every un-referenced cell.
    # Both sums come from fused scalar_tensor_tensor ops with accum_out, so
    # the slab chain is 4 DVE ops and the scalar engine only runs the tiny
    # cls tail.  Every RAW edge (even same-engine: consecutive engine
    # instructions pipeline) carries an explicit semaphore wait.
    nc.vector.tensor_tensor(dm[:], data_v, tm_v,
                            op=mybir.AluOpType.subtract)._wait_ge(
                                SA, 16).then_inc(S1, 1)            # S1=1
    nc.vector.tensor_tensor(D[:], sc_v[:, 0::2], sc_v[:, 1::2],
                            op=mybir.AluOpType.subtract)._wait_ge(
                                SB, 16).then_inc(S1, 1)            # S1=2
    nc.vector.tensor_scalar(c[:], dm[:], 1.0, -1.0,
                            mybir.AluOpType.min,
                            mybir.AluOpType.max)._wait_ge(
                                S1, 1).then_inc(S1, 1)             # S1=3
    # P0 = sum d*c:  (dm bypass) * c, accumulated
    nc.vector.scalar_tensor_tensor(dc[:], dm[:], 0.0, c[:],
                                   mybir.AluOpType.bypass,
                                   mybir.AluOpType.mult,
                                   accum_out=P[:, 0:1])._wait_ge(
                                       S1, 3).then_inc(S1, 1)      # S1=4
    # P1 = sum c^2:  (c bypass) * c, accumulated
    nc.vector.scalar_tensor_tensor(sq[:], c[:], 0.0, c[:],
                                   mybir.AluOpType.bypass,
                                   mybir.AluOpType.mult,
                                   accum_out=P[:, 1:2])._wait_ge(
                                       S1, 3).then_inc(S1, 1)      # S1=5

    # ACT queue: CE = sum softplus(D) with the mask encoded in the data
    # (un-referenced pair cells have +40 on the second logit so
    # exp(D)+1 == 1.0 and ln gives exactly 0).  then_inc on an
    # accum-activation is moved to the accumulator-read during lowering.
    nc.scalar.activation(ex[:], D[:], AF.Exp)._wait_ge(
        S1, 2).then_inc(S2, 1)                                     # S2=1
    nc.scalar.activation(ce[:], ex[:], AF.Ln, bias=1.0,
                         accum_out=P[:, 2:3])._wait_ge(
                             S2, 1).then_inc(S2, 1)                # S2=2

    # SP: ship the result; nobody waits on SO — the HBM write receipt
    # drains inside the engines' fini
    nc.sync.wait_ge(S1, 5)
    nc.sync.dma_start(OUT[:, 0:3], P[:, 0:3])._wait_ge(S2, 2).then_inc(SO, 16)
    nc.sync.sem_inc(SGATE, 1)
    # GpSimd's fini clear-chunk covers our semaphore range: hold it back
    # until the out-DMA dispatch has consumed its waits
    nc.gpsimd.wait_ge(SGATE, 1)

    nc.compile()
    return nc


def kernel(**inputs):
    score = np.asarray(inputs["score"], dtype=np.float32)[0].reshape(2 * K, HW)
    vp = np.asarray(inputs["vertical_pred"], dtype=np.float32)[0].reshape(2 * K, HW)
    side = np.asarray(inputs["side_refinement"], dtype=np.float32)[0].reshape(K, HW)
    pidx = np.asarray(inputs["positive"])
    nidx = np.asarray(inputs["negative"])
    vidx = np.asarray(inputs["vertical_reg_idx"])
    vtgt = np.asarray(inputs["vertical_reg_tgt"], dtype=np.float32)
    sidx = np.asarray(inputs["side_reg_idx"])
    stgt = np.asarray(inputs["side_reg_tgt"], dtype=np.float32)

    vp_b = _bf16(vp)        # [20, HW] uint16
    side_b = _bf16(side)    # [10, HW]
    score_b = _bf16(score)  # [20, HW]

    def fields(idx):
        x = idx[:, 0].astype(np.int64)
        y = idx[:, 1].astype(np.int64)
        a = idx[:, 2].astype(np.int64)
        pos = y * W + x
        return a, pos // PPC, pos % PPC

    va, vcore, vposl = fields(vidx)
    sa, score_, sposl = fields(sidx)
    pa, pcore, pposl = fields(pidx)
    na, ncore, nposl = fields(nidx)

    # --- sl1 cell refs: vp entries contribute 2 cells (ch 2a, 2a+1) --------
    # cell id within a core = part*F + col; vp block parts [0,80), sd [80,120)
    v_j0 = (2 * va) * PPC + vposl          # flat (ch, posl), ch-major
    v_j1 = (2 * va + 1) * PPC + vposl
    s_j = sa * PPC + sposl

    ref_core = np.concatenate([vcore, vcore, score_])
    ref_cell = np.concatenate([v_j0, v_j1, s_j + VP_PARTS * F])
    ref_tgt = np.concatenate([vtgt[:, 0], vtgt[:, 1], stgt]).astype(np.float32)
    ref_isv = np.concatenate([np.ones(2 * len(vidx), np.bool_),
                              np.zeros(len(sidx), np.bool_)])

    # --- cls pair-cell refs ------------------------------------------------
    cls_core = np.concatenate([pcore, ncore])
    cls_q = np.concatenate([pa * PPC + pposl, na * PPC + nposl])
    cls_isneg = np.concatenate([np.zeros(len(pidx), np.bool_),
                                np.ones(len(nidx), np.bool_)])

    if "nc" not in _cache:
        _cache["nc"] = _build_bass()
    nc = _cache["nc"]

    in_maps = []
    wvec_v = np.zeros((N_CORES, 128), np.float64)
    wvec_o = np.zeros((N_CORES, 128), np.float64)
    for cidx in range(N_CORES):
        sl = slice(cidx * PPC, (cidx + 1) * PPC)
        # slab data: [128, 768] uint16; vp rows then sd rows
        slab = np.zeros((128, F), np.uint16)
        slab[:VP_PARTS] = vp_b[:, sl].reshape(VP_PARTS, F)
        slab[VP_PARTS:OV0] = side_b[:, sl].reshape(SD_PARTS, F)
        tm = slab.copy()

        # weights for the regular blocks
        wvec_v[cidx, :VP_PARTS] = 1.0 / (2.0 * NV_REG)
        wvec_o[cidx, VP_PARTS:OV0] = 1.0 / NO_REG

        # write targets; duplicates go to the overflow partitions
        msel = ref_core == cidx
        cells = ref_cell[msel]
        tgts = _bf16(ref_tgt[msel])
        isv = ref_isv[msel]
        _, first = np.unique(cells, return_index=True)
        tm.reshape(-1)[cells[first]] = tgts[first]
        extra = np.ones(len(cells), np.bool_)
        extra[first] = False
        ev_cells, ev_t = cells[extra & isv], tgts[extra & isv]
        eo_cells, eo_t = cells[extra & ~isv], tgts[extra & ~isv]
        # vp extras fill overflow rows from 120 up, sd extras from 127 down
        nv_rows = (len(ev_cells) + F - 1) // F
        no_rows = (len(eo_cells) + F - 1) // F
        assert nv_rows + no_rows <= 128 - OV0, "overflow region full"
        flat_slab = slab.reshape(-1)
        if len(ev_cells):
            base = OV0 * F
            idxs = base + np.arange(len(ev_cells))
            flat_slab[idxs] = flat_slab[ev_cells]
            tm.reshape(-1)[idxs] = ev_t
            wvec_v[cidx, OV0:OV0 + nv_rows] = 1.0 / (2.0 * NV_REG)
        if len(eo_cells):
            base = 128 * F - len(eo_cells)
            idxs = base + np.arange(len(eo_cells))
            flat_slab[idxs] = flat_slab[eo_cells]
            tm.reshape(-1)[idxs] = eo_t
            wvec_o[cidx, 128 - no_rows:128] = 1.0 / NO_REG

        # --- score pairs (mask encoded in the data) -----------------------
        # pair slot q = a*3072 + posl -> (part, slot) = (q//240, q%240)
        l0f = score[0::2, sl].reshape(-1)
        l1f = score[1::2, sl].reshape(-1)
        pair = np.stack([l0f, l1f], axis=-1).copy()     # f32 [K*PPC, 2]
        cp = np.zeros(K * PPC, np.int64)
        cn = np.zeros(K * PPC, np.int64)
        csel = cls_core == cidx
        q_here = cls_q[csel]
        neg_here = cls_isneg[csel]
        np.add.at(cp, q_here[~neg_here], 1)
        np.add.at(cn, q_here[neg_here], 1)
        ref = (cp > 0) | (cn > 0)
        # un-referenced cells: bias the second logit so softplus gives 0
        pair[~ref, 1] += 40.0
        # neg-only cells: swapped orientation
        swap = (cn > 0) & (cp == 0)
        pair[swap] = pair[swap][:, ::-1]
        # multiplicity > 1 and mixed-orientation cells spill into overflow
        ov_list = []
        for q in np.nonzero((cp + cn) > 1)[0]:
            n_norm = (cp[q] - 1) if cp[q] > 0 else 0
            n_swap = cn[q] if cp[q] > 0 else (cn[q] - 1)
            ov_list += [(l0f[q], l1f[q])] * int(n_norm)
            ov_list += [(l1f[q], l0f[q])] * int(n_swap)
        assert len(ov_list) <= 128 * OVP, "cls overflow full"
        ov_pair = np.zeros((128 * OVP, 2), np.float32)
        ov_pair[:, 1] = 40.0
        if ov_list:
            ov_pair[:len(ov_list)] = np.asarray(ov_list, np.float32)

        sc_full = np.concatenate(
            [pair.reshape(128, NPAIR, 2), ov_pair.reshape(128, OVP, 2)],
            axis=1)                                      # f32 [128, 256, 2]

        mega = np.empty((128, WB), np.uint8)
        mega[:, o_data:o_tm] = slab.view(np.uint8).reshape(128, 2 * F)
        mega[:, o_tm:o_sc] = tm.view(np.uint8).reshape(128, 2 * F)
        mega[:, o_sc:WB] = _bf16(sc_full).view(np.uint8).reshape(128, 2 * SCW)
        in_maps.append({"mega": mega})

    res = bass_utils.run_bass_kernel_spmd(
        nc, in_maps, core_ids=list(range(N_CORES)))

    v_loss = 0.0
    o_loss = 0.0
    cls_sum = 0.0
    for cidx in range(N_CORES):
        P = res.results[cidx]["out"].astype(np.float64)   # [128, 4]
        S = P[:, 0] - 0.5 * P[:, 1]
        v_loss += float(np.dot(S, wvec_v[cidx]))
        o_loss += float(np.dot(S, wvec_o[cidx]))
        cls_sum += float(P[:, 2].sum())
    cls_loss = np.float32(cls_sum / NS)
    loss = np.float32(cls_loss + v_loss + o_loss)
    return (np.float32(loss), np.float32(cls_loss), np.float32(v_loss),
            np.float32(o_loss))
